# revision 12
# baseline (speedup 1.0000x reference)
"""2-layer GAT on 8 trn2 NeuronCores — edge-major blocks + TensorE one-hot
aggregation (see build_kernel2 docstring below). Self-contained: host prep +
kernel build + SPMD run + pipelined timing."""
import numpy as np

N = 100000
E = 1_600_000
NCORES = 8
RN = N // NCORES          # 12500
RROWS = RN + 1
NQ = 4
P = 128
NT = (RN + P - 1) // P    # 98
DUMMY16 = RN              # dummy row in first half of quarter table
CPOS = 4096               # positions per gather chunk (32 blocks)


def host_prep2(edge_index):
    src = np.asarray(edge_index[0], np.int64)
    dst = np.asarray(edge_index[1], np.int64)
    core_of = dst // RN

    # per-core edge lists split by quarter, sorted by dst tile
    per_core = []
    cnt = np.zeros((NCORES, NQ, NT), np.int64)
    for k in range(NCORES):
        sel = core_of == k
        s = src[sel]
        d = dst[sel] - k * RN
        q = s // (2 * RN)
        t = d // P
        order = np.lexsort((d, q))     # sort by (q, tile-implied-by-d)
        s, d, q, t = s[order], d[order], q[order], t[order]
        per_core.append((s, d, q, t))
        np.add.at(cnt[k], (q, t), 1)

    C = cnt.max(axis=0)                       # [NQ, NT] shared slot counts
    assert (C > P).all(), "tile group smaller than a block breaks straddle<=2"

    # quarter layouts
    off = np.zeros((NQ, NT), np.int64)
    Lq = np.zeros(NQ, np.int64)
    for q in range(NQ):
        off[q] = np.cumsum(np.concatenate([[0], C[q][:-1]]))
        raw = C[q].sum()
        Lq[q] = ((raw + P - 1) // P) * P
    NBq = (Lq // P).astype(np.int64)
    NBtot = int(NBq.sum())
    Ltot = int(Lq.sum())

    # shared per-position tile map (-1 = pad tail)
    tile_of = []
    for q in range(NQ):
        tm = np.full(Lq[q], -1, np.int64)
        for t in range(NT):
            tm[off[q, t]:off[q, t] + C[q, t]] = t
        tile_of.append(tm)

    # shared block->views schedule and chunking
    chunks = []       # list of dicts: q, pos0, npos, blocks:[(b, t_first, [tiles])]
    qblock0 = np.zeros(NQ, np.int64)   # global block index offset per quarter
    b0 = 0
    for q in range(NQ):
        qblock0[q] = b0
        b0 += NBq[q]
        nb = int(NBq[q])
        tm = tile_of[q]
        bviews = []
        for b in range(nb):
            ts = np.unique(tm[b * P:(b + 1) * P])
            ts = ts[ts >= 0]
            assert len(ts) <= 2, f"block straddles {len(ts)} tiles"
            if len(ts):
                assert ts[-1] - ts[0] <= 1
            bviews.append((int(tm[b * P]) if tm[b * P] >= 0 else
                           (int(ts[0]) if len(ts) else -1), [int(x) for x in ts]))
        for c0 in range(0, nb, CPOS // P):
            cb = min(CPOS // P, nb - c0)
            chunks.append(dict(q=q, b0=c0, nblk=cb,
                               pos0=int(c0 * P), npos=int(cb * P),
                               views=[(c0 + i, bviews[c0 + i][0], bviews[c0 + i][1])
                                      for i in range(cb)]))

    # per-core slot arrays
    percore = []
    for k in range(NCORES):
        s, d, q, t = per_core[k]
        g16_all = []
        dval_all = []
        for qq in range(NQ):
            g16 = np.full(Lq[qq], DUMMY16, np.int64)
            dval = np.full(Lq[qq], 999.0, np.float64)
            m = q == qq
            sq, dq, tq = s[m], d[m], t[m]
            # slot position per edge: off[qq, t] + within-group index
            # edges already sorted by (q, d): stable within-group order by d
            idxs = np.zeros(len(sq), np.int64)
            pos = 0
            # compute group start positions via counts
            cnts = np.bincount(tq, minlength=NT)
            starts = off[qq].copy()
            within = np.zeros(NT, np.int64)
            # vectorized: position = starts[t] + running index within t
            run = np.concatenate([[0], np.cumsum(cnts)[:-1]])
            order_in_group = np.arange(len(sq)) - run[tq]
            slot = starts[tq] + order_in_group
            g16[slot] = ((sq // RN) % 2) * RROWS + (sq % RN)
            tfirst_of_pos = np.repeat(
                [bviews_tfirst for bviews_tfirst in []], 0)  # placeholder
            # dval relative to the block's first tile
            blk = slot // P
            tfirst = np.array([tile_of[qq][b * P] for b in range(int(NBq[qq]))])
            # pad-tail blocks: tm[b*P] may be -1 -> no views; edges never land there
            dval[slot] = dq - P * tfirst[blk]
            g16_all.append(g16)
            dval_all.append(dval)
        g16_all = np.concatenate(g16_all)
        dval_all = np.concatenate(dval_all)
        percore.append(dict(g16=g16_all.astype(np.int16),
                            dval=dval_all.astype(np.float32)))

    meta = dict(C=C, off=off, Lq=Lq, NBq=NBq, NBtot=NBtot, Ltot=Ltot,
                chunks=chunks, tile_of=tile_of, qblock0=qblock0)
    return meta, percore




import hashlib
import numpy as np
import ml_dtypes

from concourse import bass, mybir, tile, bacc, bass_utils

f32 = mybir.dt.float32
bf16 = mybir.dt.bfloat16
i16 = mybir.dt.int16

F_IN = 512
H1, C1 = 8, 8
F1 = H1 * C1            # 64
C2 = 32
NEG = 0.2
BF = ml_dtypes.bfloat16

_cache = {}
_last_in_maps = None


def _wrap_idx(arr):
    """dma_gather index layout: [128, L/16] int16, wrapped by 16, replicated."""
    a = np.asarray(arr, np.int16)
    assert a.size % 16 == 0
    w = a.reshape(-1, 16).T
    return np.tile(w, (8, 1)).copy()


def dma_gather_raw(nc, out_ap, in_ap, idxs_ap, num_idxs, elem_size, elem_step):
    """bass dma_gather without the elem%256 assert (stride must be %256B)."""
    gp = nc.gpsimd
    stride_bytes = elem_step * mybir.dt.size(in_ap.dtype)
    assert stride_bytes % 256 == 0
    _in_ap = gp.lower_ap_dma(in_ap, for_custom_bir_dma=True)
    _idxs_ap = gp.lower_ap(idxs_ap)
    _out_ap = gp.lower_ap(out_ap)
    return gp.add_instruction(mybir.InstDMAGatherAnt(
        name=nc.get_next_instruction_name(),
        ins=[*_in_ap, _idxs_ap, gp.lower_val_access(gp.to_reg(num_idxs))],
        outs=[_out_ap],
        transpose=False, num_idxs=num_idxs, elem_size=elem_size,
        stride_bytes_256=stride_bytes // 256, gen_mode=0,
        single_packet=False, queue_num=0,
        sbuf_tokens_per_rank=0, sbuf_free_dim_per_rank=0,
        sbuf_free_dim_pad_per_rank=0, sbuf_byte_offset=0))


def build_kernel2(meta, stage='full'):
    chunks = meta["chunks"]
    Lq = meta["Lq"]
    qblock0 = meta["qblock0"]
    NBtot = int(meta["NBtot"])
    Ltot = int(meta["Ltot"])
    qpos0 = np.concatenate([[0], np.cumsum(Lq)]).astype(np.int64)

    nc = bacc.Bacc("TRN2", target_bir_lowering=False, debug=False,
                   enable_asserts=False, num_devices=NCORES)
    AL = mybir.AluOpType
    AX = mybir.AxisListType
    ACT = mybir.ActivationFunctionType

    xT = nc.dram_tensor("xT", [F_IN, RN], bf16, kind="ExternalInput")
    W1a = nc.dram_tensor("W1a", [F_IN, 80], bf16, kind="ExternalInput")
    W2a = nc.dram_tensor("W2a", [F1, 34], bf16, kind="ExternalInput")
    cvec = nc.dram_tensor("cvec", [P, 34], f32, kind="ExternalInput")
    t1i = nc.dram_tensor("t1i", [P, Ltot // 16], i16, kind="ExternalInput")
    dcol_d = nc.dram_tensor("dcol", [P, NBtot], bf16, kind="ExternalInput")
    drow_d = nc.dram_tensor("drow", [1, NBtot * P], bf16, kind="ExternalInput")
    iotac = nc.dram_tensor("iotac", [P, 256], bf16, kind="ExternalInput")
    iotap = nc.dram_tensor("iotap", [P, 2], f32, kind="ExternalInput")
    out = nc.dram_tensor("out", [RN, C2], f32, kind="ExternalOutput")

    run_l1 = stage in ('l1g', 'l1', 'p1', 'ag2', 'l2g', 'l2', 'full')
    run_p1 = stage in ('p1', 'ag2', 'l2g', 'l2', 'full')
    run_ag2 = stage in ('ag2', 'l2g', 'l2', 'full')
    run_l2 = stage in ('l2g', 'l2', 'full')
    run_p2 = stage in ('full',)
    run_ag1 = stage in ('ag',) or run_l1

    with tile.TileContext(nc) as tc:
        with tc.tile_pool(name="dram", bufs=1, space="DRAM") as dram, \
             tc.tile_pool(name="const", bufs=1) as cpool:
            T1c = dram.tile([RROWS, 128], bf16)
            T1f = dram.tile([NCORES * RROWS, 128], bf16, addr_space="Shared")
            T2c = dram.tile([RROWS, 128], bf16)
            T2f = dram.tile([NCORES * RROWS, 128], bf16, addr_space="Shared")

            from concourse.masks import make_identity
            ident = cpool.tile([P, P], f32)
            make_identity(nc, ident[:])
            w2sb = cpool.tile([F1, 34], bf16)
            nc.sync.dma_start(w2sb[:], W2a[:])
            cvr = cpool.tile([P, 34], f32)
            nc.sync.dma_start(cvr[:], cvec[:])
            iot = cpool.tile([P, 256], bf16)     # [:,0:128]=j, [:,128:256]=j+128
            nc.sync.dma_start(iot[:], iotac[:])
            iop = cpool.tile([P, 2], f32)        # [:,0]=p, [:,1]=p+128
            nc.sync.dma_start(iop[:], iotap[:])
            dummy1 = cpool.tile([1, 128], bf16)
            nc.vector.memset(dummy1[:, 0:64], 0.0)
            nc.vector.memset(dummy1[:, 64:128], -1e30)
            dummy2 = cpool.tile([1, 128], bf16)
            nc.vector.memset(dummy2[:, 0:32], 0.0)
            nc.vector.memset(dummy2[:, 32:128], -1e30)
            nc.sync.dma_start(T1c[RN:RN + 1, :], dummy1[:])
            nc.sync.dma_start(T2c[RN:RN + 1, :], dummy2[:])

            idx_res = cpool.tile([P, Ltot // 16], i16)
            nc.sync.dma_start(idx_res[:], t1i[:])
            dcol = cpool.tile([P, NBtot], bf16)
            nc.sync.dma_start(dcol[:], dcol_d[:])
            dcolf = cpool.tile([P, NBtot], f32)   # f32 copy for ts scalars
            nc.vector.tensor_copy(dcolf[:], dcol[:])

            adt1 = cpool.tile([P, NT, 8], bf16)
            nc.vector.memset(adt1[:].rearrange("p t e -> p (t e)"), 0.0)
            adt2 = cpool.tile([P, NT, 1], bf16)
            nc.vector.memset(adt2[:].rearrange("p t e -> p (t e)"), 0.0)
            nd1 = cpool.tile([P, NT, 72], f32)
            nc.vector.memset(nd1[:].rearrange("p t e -> p (t e)"), 0.0)
            nd2 = cpool.tile([P, NT, 33], f32)
            nc.vector.memset(nd2[:].rearrange("p t e -> p (t e)"), 0.0)

            # ---------- phase 1: h_aug = x @ W1aug ----------
            with tc.tile_pool(name="mmx", bufs=2) as xpool, \
                 tc.tile_pool(name="mmw", bufs=4) as wpool, \
                 tc.tile_pool(name="mmp", bufs=8, space="PSUM") as pspool, \
                 tc.tile_pool(name="mmo", bufs=4) as opool:
                w1sb = []
                for kc in range(4):
                    wt_ = wpool.tile([P, 80], bf16, tag=f"w1_{kc}")
                    nc.sync.dma_start(wt_[:], W1a[kc * P:(kc + 1) * P, :])
                    w1sb.append(wt_)
                BT = 8
                for b0 in range(0, NT, BT):
                    bts = list(range(b0, min(b0 + BT, NT)))
                    c0 = b0 * P
                    c1 = min(bts[-1] * P + P, RN)
                    ps = {t: pspool.tile([P, 80], f32, space="PSUM", tag="ps",
                                         name=f"ps_{t}")
                          for t in bts}
                    for kc in range(4):
                        xs = xpool.tile([P, BT * P], bf16, tag="xs")
                        nc.sync.dma_start(xs[:, 0:c1 - c0],
                                          xT[kc * P:(kc + 1) * P, c0:c1])
                        for t in bts:
                            m = min(P, RN - t * P)
                            nc.tensor.matmul(
                                ps[t][0:m, :],
                                lhsT=xs[:, t * P - c0:t * P - c0 + m],
                                rhs=w1sb[kc][:], start=(kc == 0), stop=(kc == 3))
                    for t in bts:
                        m = min(P, RN - t * P)
                        r1 = opool.tile([P, 72], bf16, tag="r1")
                        nc.vector.tensor_copy(r1[0:m, :], ps[t][0:m, 0:72])
                        nc.sync.dma_start(
                            T1c[t * P:t * P + m, 0:72], r1[0:m, :])
                        # alpha_dst resident
                        nc.vector.tensor_copy(adt1[0:m, t, :], ps[t][0:m, 72:80])
                        # self-loop init of nd1
                        asd = opool.tile([P, 8], f32, tag="asd")
                        nc.vector.tensor_tensor(
                            out=asd[0:m, :], in0=ps[t][0:m, 64:72],
                            in1=adt1[0:m, t, :], op=AL.add)
                        tmp = opool.tile([P, 8], f32, tag="lr")
                        nc.vector.tensor_scalar(
                            out=tmp[0:m, :], in0=asd[0:m, :], scalar1=NEG,
                            scalar2=None, op0=AL.mult)
                        nc.vector.tensor_tensor(
                            out=asd[0:m, :], in0=asd[0:m, :], in1=tmp[0:m, :],
                            op=AL.max)
                        nc.scalar.activation(nd1[0:m, t, 64:72], asd[0:m, :],
                                             ACT.Exp)
                        nc.vector.tensor_tensor(
                            out=nd1[0:m, t, 0:64].rearrange(
                                "p (h c) -> p h c", c=8),
                            in0=ps[t][0:m, 0:64].rearrange(
                                "p (h c) -> p h c", c=8),
                            in1=nd1[0:m, t, 64:72].unsqueeze(2)
                            .broadcast_to([m, 8, 8]),
                            op=AL.mult)

            # ---------- AllGather T1 ----------
            if run_ag1:
                nc.gpsimd.collective_compute(
                    "AllGather", AL.bypass, ins=[T1c.opt()], outs=[T1f.opt()],
                    replica_groups=[list(range(NCORES))])

            # ---------- edge phase ----------
            def edge_layer(layer, Ttbl, ndT, adT, gather_only):
                FEAT = F1 if layer == 1 else C2       # 64 / 32
                NH = H1 if layer == 1 else 1
                EL = FEAT + NH                        # 72 / 33 gathered elems
                MW = FEAT + NH                        # message width
                ELG = 72 if layer == 1 else 34        # table row payload
                with tc.tile_pool(name=f"eg{layer}", bufs=3) as eg, \
                     tc.tile_pool(name=f"es{layer}", bufs=2) as es, \
                     tc.tile_pool(name=f"em{layer}", bufs=2) as em, \
                     tc.tile_pool(name=f"ep{layer}", bufs=2, space="PSUM") as epp, \
                     tc.tile_pool(name=f"ea{layer}", bufs=4, space="PSUM") as eap:
                    for ch in chunks:
                        q = ch["q"]
                        nblk = ch["nblk"]
                        npos = ch["npos"]
                        posg = int(qpos0[q] + ch["pos0"])
                        blkg = int(qblock0[q] + ch["b0"])
                        eb = eg.tile([P, nblk, ELG], bf16, tag="eb")
                        dma_gather_raw(
                            nc, eb[:],
                            Ttbl[2 * RROWS * q:2 * RROWS * (q + 1), 0:ELG],
                            idx_res[:, posg // 16:(posg + npos) // 16],
                            npos, ELG, 128)
                        drw = eg.tile([P, nblk * P], bf16, tag="drw")
                        nc.sync.dma_start(
                            drw[:],
                            drow_d[0:1, blkg * P:(blkg + nblk) * P]
                            .broadcast_to([P, nblk * P]))
                        if gather_only:
                            continue
                        # S_T (n-part, e-free), base-tile orientation.
                        # tensor_scalar with a per-partition scalar keeps the
                        # streaming operand contiguous -> 4x DVE mode.
                        st0 = es.tile([P, nblk, P], bf16, tag="st0")
                        nc.vector.tensor_scalar(
                            out=st0[:].rearrange("p b e -> p (b e)"),
                            in0=drw[:], scalar1=iop[:, 0:1], scalar2=None,
                            op0=AL.is_equal)
                        # straddle S_T (second tile): same compare vs p+128,
                        # batched for the whole chunk
                        has_straddle = any(len(tiles) == 2
                                           for (_, _, tiles) in ch["views"])
                        if has_straddle:
                            st0b = es.tile([P, nblk, P], bf16, tag="st0b")
                            nc.vector.tensor_scalar(
                                out=st0b[:].rearrange("p b e -> p (b e)"),
                                in0=drw[:], scalar1=iop[:, 1:2], scalar2=None,
                                op0=AL.is_equal)
                        # alpha_dst per edge via S_T matmuls
                        adp = eap.tile([P, nblk, NH], f32, space="PSUM",
                                       tag="adp")
                        for (b, tf, tiles) in ch["views"]:
                            ib = b - ch["b0"]
                            for j, t in enumerate(tiles):
                                lhsT = (st0[:, ib, :] if t == tf
                                        else st0b[:, ib, :])
                                nc.tensor.matmul(
                                    adp[:, ib, :], lhsT=lhsT,
                                    rhs=adT[:, t, :],
                                    start=(j == 0), stop=(j == len(tiles) - 1))
                        # alpha -> lrelu -> exp
                        wq = em.tile([P, nblk, NH], f32, tag="wq")
                        nc.vector.tensor_tensor(
                            out=wq[:], in0=eb[:, :, FEAT:FEAT + NH],
                            in1=adp[:], op=AL.add)
                        tmp = em.tile([P, nblk * NH], f32, tag="lrt")
                        nc.vector.tensor_scalar(
                            out=tmp[:], in0=wq[:].rearrange("p b h -> p (b h)"),
                            scalar1=NEG, scalar2=None, op0=AL.mult)
                        nc.vector.tensor_tensor(
                            out=wq[:].rearrange("p b h -> p (b h)"),
                            in0=wq[:].rearrange("p b h -> p (b h)"),
                            in1=tmp[:], op=AL.max)
                        nc.scalar.activation(
                            wq[:].rearrange("p b h -> p (b h)"),
                            wq[:].rearrange("p b h -> p (b h)"), ACT.Exp)
                        # messages
                        msg = em.tile([P, nblk, MW], bf16, tag="msg")
                        nc.vector.tensor_copy(msg[:, :, FEAT:MW], wq[:])
                        if layer == 1:
                            nc.vector.tensor_tensor(
                                out=msg[:, :, 0:64].rearrange(
                                    "p b (h c) -> p b h c", c=8),
                                in0=eb[:, :, 0:64].rearrange(
                                    "p b (h c) -> p b h c", c=8),
                                in1=wq[:].unsqueeze(3)
                                .broadcast_to([P, nblk, 8, 8]),
                                op=AL.mult)
                        else:
                            nc.vector.tensor_tensor(
                                out=msg[:, :, 0:32],
                                in0=eb[:, :, 0:32],
                                in1=wq[:].broadcast_to([P, nblk, 32]),
                                op=AL.mult)
                        # S (e-part, n-free)
                        s0 = es.tile([P, nblk, P], bf16, tag="s0")
                        nc.vector.tensor_tensor(
                            out=s0[:],
                            in0=dcol[:, blkg:blkg + nblk].unsqueeze(2)
                            .broadcast_to([P, nblk, P]),
                            in1=iot[:, 0:128].unsqueeze(1)
                            .broadcast_to([P, nblk, P]),
                            op=AL.is_equal)
                        s1 = {}
                        for (b, tf, tiles) in ch["views"]:
                            if len(tiles) == 2:
                                ib = b - ch["b0"]
                                s = es.tile([P, P], bf16, tag="s1",
                                            name=f"s1_{layer}_{blkg}_{ib}")
                                nc.vector.tensor_scalar(
                                    out=s[:], in0=iot[:, 128:256],
                                    scalar1=dcolf[:, blkg + ib:blkg + ib + 1],
                                    scalar2=None, op0=AL.is_equal)
                                s1[ib] = s
                        # aggregate per tile
                        tviews = {}
                        for (b, tf, tiles) in ch["views"]:
                            ib = b - ch["b0"]
                            for t in tiles:
                                tviews.setdefault(t, []).append(
                                    (ib, t == tf))
                        for t in sorted(tviews):
                            vs = tviews[t]
                            pst = epp.tile([P, MW], f32, space="PSUM",
                                           tag="aggps")
                            for j, (ib, base) in enumerate(vs):
                                lhsT = s0[:, ib, :] if base else s1[ib][:]
                                nc.tensor.matmul(
                                    pst[:], lhsT=lhsT, rhs=msg[:, ib, :],
                                    start=(j == 0), stop=(j == len(vs) - 1))
                            nc.vector.tensor_tensor(
                                out=ndT[:, t, :], in0=ndT[:, t, :],
                                in1=pst[:], op=AL.add)

            if run_l1:
                edge_layer(1, T1f, nd1, adt1, stage == 'l1g')

            # ---------- post 1: normalize, ELU+1, W2 -> T2c ----------
            if run_p1:
                EC = 10
                with tc.tile_pool(name="p1", bufs=2) as ep, \
                     tc.tile_pool(name="p1p", bufs=4, space="PSUM") as epp:
                    for e0 in range(0, NT, EC):
                        Tc = min(EC, NT - e0)
                        # +eps so the tail tile's empty rows give 0*1e30=0
                        # instead of 0*inf=NaN (which would poison layer-2
                        # matmuls through adt2 via 0*NaN).
                        reci = ep.tile([P, Tc, 8], f32, tag="reci")
                        nc.vector.tensor_scalar(
                            out=reci[:], in0=nd1[:, e0:e0 + Tc, 64:72],
                            scalar1=1e-30, scalar2=None, op0=AL.add)
                        rec = ep.tile([P, Tc, 8], f32, tag="rec")
                        nc.vector.reciprocal(rec[:], reci[:])
                        h1p = ep.tile([P, Tc, 64], f32, tag="h1p")
                        nc.vector.tensor_tensor(
                            out=h1p[:].rearrange("p t (h c) -> p t h c", c=8),
                            in0=nd1[:, e0:e0 + Tc, 0:64].rearrange(
                                "p t (h c) -> p t h c", c=8),
                            in1=rec[:].unsqueeze(3).broadcast_to([P, Tc, 8, 8]),
                            op=AL.mult)
                        # ELU + 1 = relu(x) + exp(min(x, 0))
                        rl = ep.tile([P, Tc * 64], f32, tag="rl")
                        h1f = h1p[:].rearrange("p t f -> p (t f)")
                        nc.scalar.activation(rl[:], h1f, ACT.Relu)
                        nc.vector.tensor_tensor(out=h1f, in0=h1f, in1=rl[:],
                                                op=AL.subtract)
                        nc.scalar.activation(h1f, h1f, ACT.Exp)
                        nc.vector.tensor_tensor(out=h1f, in0=h1f, in1=rl[:],
                                                op=AL.add)
                        t2a = ep.tile([P, Tc, 34], f32, tag="t2a")
                        for i in range(Tc):
                            tps = epp.tile([64, P], f32, space="PSUM",
                                           tag="tps")
                            nc.tensor.transpose(tps[:], h1p[:, i, :], ident[:])
                            hT = ep.tile([64, P], bf16, tag="hT")
                            nc.vector.tensor_copy(hT[:], tps[:])
                            ps2 = epp.tile([P, 34], f32, space="PSUM",
                                           tag="ps2")
                            nc.tensor.matmul(ps2[:], lhsT=hT[:], rhs=w2sb[:],
                                             start=True, stop=True)
                            nc.vector.tensor_tensor(
                                out=t2a[:, i, :], in0=ps2[:], in1=cvr[:],
                                op=AL.add)
                        # alpha_dst2 resident + self init of nd2
                        nc.vector.tensor_copy(
                            adt2[:, e0:e0 + Tc, 0], t2a[:, :, 33])
                        asd = ep.tile([P, Tc], f32, tag="asd2")
                        nc.vector.tensor_tensor(
                            out=asd[:], in0=t2a[:, :, 32], in1=t2a[:, :, 33],
                            op=AL.add)
                        tmp = ep.tile([P, Tc], f32, tag="lr2")
                        nc.vector.tensor_scalar(
                            out=tmp[:], in0=asd[:], scalar1=NEG, scalar2=None,
                            op0=AL.mult)
                        nc.vector.tensor_tensor(out=asd[:], in0=asd[:],
                                                in1=tmp[:], op=AL.max)
                        nc.scalar.activation(nd2[:, e0:e0 + Tc, 32], asd[:],
                                             ACT.Exp)
                        nc.vector.tensor_tensor(
                            out=nd2[:, e0:e0 + Tc, 0:32],
                            in0=t2a[:, :, 0:32],
                            in1=nd2[:, e0:e0 + Tc, 32:33]
                            .broadcast_to([P, Tc, 32]),
                            op=AL.mult)
                        t2sb = ep.tile([P, Tc, 34], bf16, tag="t2sb")
                        nc.vector.tensor_copy(
                            t2sb[:].rearrange("p t e -> p (t e)"),
                            t2a[:].rearrange("p t e -> p (t e)"))
                        nfull = min((e0 + Tc) * P, RN) - e0 * P
                        tfull = nfull // P
                        if tfull:
                            nc.sync.dma_start(
                                T2c[e0 * P:e0 * P + tfull * P, 0:34]
                                .rearrange("(t p) e -> p t e", p=P),
                                t2sb[:, 0:tfull, :])
                        rem = nfull - tfull * P
                        if rem:
                            nc.sync.dma_start(
                                T2c[e0 * P + tfull * P:
                                    e0 * P + tfull * P + rem, 0:34],
                                t2sb[0:rem, tfull, :])


            # ---------- AllGather T2 + layer 2 ----------
            if run_ag2:
                nc.gpsimd.collective_compute(
                    "AllGather", AL.bypass, ins=[T2c.opt()], outs=[T2f.opt()],
                    replica_groups=[list(range(NCORES))])
            if run_l2:
                edge_layer(2, T2f, nd2, adt2, stage == 'l2g')

            # ---------- post 2: log_softmax -> out ----------
            if run_p2:
                EC = 10
                with tc.tile_pool(name="p2", bufs=2) as ep:
                    for e0 in range(0, NT, EC):
                        Tc = min(EC, NT - e0)
                        rc2i = ep.tile([P, Tc, 1], f32, tag="rc2i")
                        nc.vector.tensor_scalar(
                            out=rc2i[:], in0=nd2[:, e0:e0 + Tc, 32:33],
                            scalar1=1e-30, scalar2=None, op0=AL.add)
                        rec2 = ep.tile([P, Tc, 1], f32, tag="rec2")
                        nc.vector.reciprocal(rec2[:], rc2i[:])
                        lg = ep.tile([P, Tc, 32], f32, tag="lg")
                        nc.vector.tensor_tensor(
                            out=lg[:], in0=nd2[:, e0:e0 + Tc, 0:32],
                            in1=rec2[:].broadcast_to([P, Tc, 32]), op=AL.mult)
                        mx = ep.tile([P, Tc], f32, tag="mx")
                        nc.vector.tensor_reduce(out=mx[:], in_=lg[:],
                                                axis=AX.X, op=AL.max)
                        nc.vector.tensor_tensor(
                            out=lg[:], in0=lg[:],
                            in1=mx[:].unsqueeze(2).broadcast_to([P, Tc, 32]),
                            op=AL.subtract)
                        ex = ep.tile([P, Tc, 32], f32, tag="ex")
                        nc.scalar.activation(
                            ex[:].rearrange("p t c -> p (t c)"),
                            lg[:].rearrange("p t c -> p (t c)"), ACT.Exp)
                        sm = ep.tile([P, Tc], f32, tag="sm")
                        nc.vector.tensor_reduce(out=sm[:], in_=ex[:],
                                                axis=AX.X, op=AL.add)
                        nc.scalar.activation(sm[:], sm[:], ACT.Ln)
                        nc.vector.tensor_tensor(
                            out=lg[:], in0=lg[:],
                            in1=sm[:].unsqueeze(2).broadcast_to([P, Tc, 32]),
                            op=AL.subtract)
                        nfull = min((e0 + Tc) * P, RN) - e0 * P
                        tfull = nfull // P
                        if tfull:
                            nc.sync.dma_start(
                                out[e0 * P:e0 * P + tfull * P, :]
                                .rearrange("(t p) e -> p t e", p=P),
                                lg[:, 0:tfull, :])
                        rem = nfull - tfull * P
                        if rem:
                            nc.sync.dma_start(
                                out[e0 * P + tfull * P:
                                    e0 * P + tfull * P + rem, :],
                                lg[0:rem, tfull, :])

    nc.finalize()
    return nc


def make_in_maps(meta, percore, x, W1, a1s, a1d, W2, a2s, a2d):
    A1s = np.zeros((F1, H1), np.float32)
    A1d = np.zeros((F1, H1), np.float32)
    for h in range(H1):
        A1s[h * C1:(h + 1) * C1, h] = a1s[h]
        A1d[h * C1:(h + 1) * C1, h] = a1d[h]
    W1aug = np.concatenate([W1, W1 @ A1s, W1 @ A1d], axis=1)          # [512,80]
    W2aug = np.concatenate([W2, W2 @ a2s.reshape(C2, 1),
                            W2 @ a2d.reshape(C2, 1)], axis=1)          # [64,34]
    cvecv = np.tile((-W2aug.sum(axis=0, dtype=np.float64))
                    .astype(np.float32).reshape(1, 34), (P, 1))
    xTf = np.ascontiguousarray(x.T).astype(BF)
    W1a_bf = W1aug.astype(BF)
    W2a_bf = W2aug.astype(BF)
    iotac = np.tile(np.arange(256, dtype=np.float32).reshape(1, 256),
                    (P, 1)).astype(BF)
    iotap = np.stack([np.arange(P, dtype=np.float32),
                      np.arange(P, dtype=np.float32) + 128],
                     axis=1)
    in_maps = []
    for k in range(NCORES):
        pc = percore[k]
        g16 = pc["g16"]
        dval = pc["dval"].astype(BF)
        NB = len(g16) // P
        in_maps.append(dict(
            xT=np.ascontiguousarray(xTf[:, k * RN:(k + 1) * RN]),
            W1a=W1a_bf, W2a=W2a_bf, cvec=cvecv,
            t1i=_wrap_idx(g16),
            dcol=np.ascontiguousarray(dval.reshape(NB, P).T),
            drow=dval.reshape(1, -1),
            iotac=iotac, iotap=iotap))
    return in_maps


def kernel(**inputs):
    x = np.asarray(inputs["x"], np.float32)
    edge_index = np.asarray(inputs["edge_index"])
    W1 = np.asarray(inputs["W1"], np.float32)
    a1s = np.asarray(inputs["att_src1"], np.float32)
    a1d = np.asarray(inputs["att_dst1"], np.float32)
    W2 = np.asarray(inputs["W2"], np.float32)
    a2s = np.asarray(inputs["att_src2"], np.float32)
    a2d = np.asarray(inputs["att_dst2"], np.float32)

    key = hashlib.sha1(edge_index.tobytes()).hexdigest()
    if key not in _cache:
        meta, percore = host_prep2(edge_index)
        nc = build_kernel2(meta)
        _cache[key] = (meta, percore, nc)
    meta, percore, nc = _cache[key]

    in_maps = make_in_maps(meta, percore, x, W1, a1s, a1d, W2, a2s, a2d)
    global _last_in_maps
    _last_in_maps = in_maps
    res = bass_utils.run_bass_kernel_spmd(nc, in_maps,
                                          core_ids=list(range(NCORES)))
    return np.concatenate([np.asarray(res.results[k]["out"])
                           for k in range(NCORES)], axis=0)


def timed_run(n=5, depth=96):
    """Pipelined per-execution NEFF time (amortizes the host round-trip)."""
    import time
    import jax
    from jax.sharding import Mesh, PartitionSpec
    from jax.experimental.shard_map import shard_map
    from concourse import bass2jax, mybir as mb

    meta, percore, nc = next(iter(_cache.values()))
    in_maps = _last_in_maps
    bass2jax.install_neuronx_cc_hook()
    in_names, out_names, out_avals, zero_outs = [], [], [], []
    for alloc in nc.m.functions[0].allocations:
        if not isinstance(alloc, mb.MemoryLocationSet):
            continue
        name = alloc.memorylocations[0].name
        pid_name = nc.partition_id_tensor.name if nc.partition_id_tensor else None
        if alloc.kind == "ExternalInput":
            if name != pid_name:
                in_names.append(name)
        elif alloc.kind == "ExternalOutput":
            out_names.append(name)
            shape = tuple(alloc.tensor_shape)
            dtype = mb.dt.np(alloc.dtype)
            out_avals.append(jax.core.ShapedArray(shape, dtype))
            zero_outs.append(np.zeros(shape, dtype))
    n_params = len(in_names)
    all_names = in_names + out_names
    if nc.partition_id_tensor is not None:
        all_names = all_names + [nc.partition_id_tensor.name]

    def _body(*args):
        ops = list(args)
        if nc.partition_id_tensor is not None:
            ops.append(bass2jax.partition_id_tensor())
        outs = bass2jax._bass_exec_p.bind(
            *ops, out_avals=tuple(out_avals), in_names=tuple(all_names),
            out_names=tuple(out_names), lowering_input_output_aliases=(),
            sim_require_finite=True, sim_require_nnan=True, nc=nc)
        return tuple(outs)

    devices = jax.devices()[:NCORES]
    mesh = Mesh(np.asarray(devices), ("core",))
    nin = n_params + len(out_names)
    sm = shard_map(_body, mesh=mesh,
                   in_specs=(PartitionSpec("core"),) * nin,
                   out_specs=(PartitionSpec("core"),) * len(out_names),
                   check_rep=False)
    fn = jax.jit(sm, keep_unused=True)
    concat_in = [np.concatenate([np.asarray(in_maps[c][nm])
                                 for c in range(NCORES)]) for nm in in_names]
    concat_zero = [np.zeros((NCORES * z.shape[0], *z.shape[1:]), z.dtype)
                   for z in zero_outs]
    sh = jax.sharding.NamedSharding(mesh, PartitionSpec("core"))
    dev_in = [jax.device_put(a, sh) for a in concat_in + concat_zero]
    outw = fn(*dev_in)
    jax.block_until_ready(outw)
    ts = []
    for _ in range(n):
        t0 = time.perf_counter()
        outs = [fn(*dev_in) for _ in range(depth)]
        jax.block_until_ready(outs)
        ts.append((time.perf_counter() - t0) / depth)
    return min(ts) * 1e9


# revision 13
# speedup vs baseline: 1.0510x; 1.0510x over previous
"""2-layer GAT on 8 trn2 NeuronCores - edge-major + TensorE one-hot aggregation."""
import numpy as np

N = 100000
E = 1_600_000
NCORES = 8
RN = N // NCORES          # 12500
RROWS = RN + 1
NQ = 4
P = 128
NT = (RN + P - 1) // P    # 98
DUMMY16 = RN              # dummy row in first half of quarter table
CPOS = 4096               # positions per gather chunk (32 blocks)


def host_prep2(edge_index):
    src = np.asarray(edge_index[0], np.int64)
    dst = np.asarray(edge_index[1], np.int64)
    core_of = dst // RN

    # per-core edge lists split by quarter, sorted by dst tile
    per_core = []
    cnt = np.zeros((NCORES, NQ, NT), np.int64)
    for k in range(NCORES):
        sel = core_of == k
        s = src[sel]
        d = dst[sel] - k * RN
        q = s // (2 * RN)
        t = d // P
        order = np.lexsort((d, q))     # sort by (q, tile-implied-by-d)
        s, d, q, t = s[order], d[order], q[order], t[order]
        per_core.append((s, d, q, t))
        np.add.at(cnt[k], (q, t), 1)

    C = cnt.max(axis=0)                       # [NQ, NT] shared slot counts
    assert (C > P).all(), "tile group smaller than a block breaks straddle<=2"

    # quarter layouts
    off = np.zeros((NQ, NT), np.int64)
    Lq = np.zeros(NQ, np.int64)
    for q in range(NQ):
        off[q] = np.cumsum(np.concatenate([[0], C[q][:-1]]))
        raw = C[q].sum()
        Lq[q] = ((raw + P - 1) // P) * P
    NBq = (Lq // P).astype(np.int64)
    NBtot = int(NBq.sum())
    Ltot = int(Lq.sum())

    # shared per-position tile map (-1 = pad tail)
    tile_of = []
    for q in range(NQ):
        tm = np.full(Lq[q], -1, np.int64)
        for t in range(NT):
            tm[off[q, t]:off[q, t] + C[q, t]] = t
        tile_of.append(tm)

    # shared block->views schedule and chunking
    chunks = []       # list of dicts: q, pos0, npos, blocks:[(b, t_first, [tiles])]
    qblock0 = np.zeros(NQ, np.int64)   # global block index offset per quarter
    b0 = 0
    for q in range(NQ):
        qblock0[q] = b0
        b0 += NBq[q]
        nb = int(NBq[q])
        tm = tile_of[q]
        bviews = []
        for b in range(nb):
            ts = np.unique(tm[b * P:(b + 1) * P])
            ts = ts[ts >= 0]
            assert len(ts) <= 2, f"block straddles {len(ts)} tiles"
            if len(ts):
                assert ts[-1] - ts[0] <= 1
            bviews.append((int(tm[b * P]) if tm[b * P] >= 0 else
                           (int(ts[0]) if len(ts) else -1), [int(x) for x in ts]))
        for c0 in range(0, nb, CPOS // P):
            cb = min(CPOS // P, nb - c0)
            chunks.append(dict(q=q, b0=c0, nblk=cb,
                               pos0=int(c0 * P), npos=int(cb * P),
                               views=[(c0 + i, bviews[c0 + i][0], bviews[c0 + i][1])
                                      for i in range(cb)]))

    # per-core slot arrays
    percore = []
    for k in range(NCORES):
        s, d, q, t = per_core[k]
        g16_all = []
        dval_all = []
        for qq in range(NQ):
            g16 = np.full(Lq[qq], DUMMY16, np.int64)
            dval = np.full(Lq[qq], 999.0, np.float64)
            m = q == qq
            sq, dq, tq = s[m], d[m], t[m]
            # slot position per edge: off[qq, t] + within-group index
            # edges already sorted by (q, d): stable within-group order by d
            idxs = np.zeros(len(sq), np.int64)
            pos = 0
            # compute group start positions via counts
            cnts = np.bincount(tq, minlength=NT)
            starts = off[qq].copy()
            within = np.zeros(NT, np.int64)
            # vectorized: position = starts[t] + running index within t
            run = np.concatenate([[0], np.cumsum(cnts)[:-1]])
            order_in_group = np.arange(len(sq)) - run[tq]
            slot = starts[tq] + order_in_group
            g16[slot] = ((sq // RN) % 2) * RROWS + (sq % RN)
            tfirst_of_pos = np.repeat(
                [bviews_tfirst for bviews_tfirst in []], 0)  # placeholder
            # dval relative to the block's first tile
            blk = slot // P
            tfirst = np.array([tile_of[qq][b * P] for b in range(int(NBq[qq]))])
            # pad-tail blocks: tm[b*P] may be -1 -> no views; edges never land there
            dval[slot] = dq - P * tfirst[blk]
            g16_all.append(g16)
            dval_all.append(dval)
        g16_all = np.concatenate(g16_all)
        dval_all = np.concatenate(dval_all)
        percore.append(dict(g16=g16_all.astype(np.int16),
                            dval=dval_all.astype(np.float32)))

    meta = dict(C=C, off=off, Lq=Lq, NBq=NBq, NBtot=NBtot, Ltot=Ltot,
                chunks=chunks, tile_of=tile_of, qblock0=qblock0)
    return meta, percore




import hashlib
import numpy as np
import ml_dtypes

from concourse import bass, mybir, tile, bacc, bass_utils

f32 = mybir.dt.float32
bf16 = mybir.dt.bfloat16
i16 = mybir.dt.int16

F_IN = 512
H1, C1 = 8, 8
F1 = H1 * C1            # 64
C2 = 32
NEG = 0.2
BF = ml_dtypes.bfloat16

_cache = {}
_last_in_maps = None


def _wrap_idx(arr):
    """dma_gather index layout: [128, L/16] int16, wrapped by 16, replicated."""
    a = np.asarray(arr, np.int16)
    assert a.size % 16 == 0
    w = a.reshape(-1, 16).T
    return np.tile(w, (8, 1)).copy()


def dma_gather_raw(nc, out_ap, in_ap, idxs_ap, num_idxs, elem_size, elem_step):
    """bass dma_gather without the elem%256 assert (stride must be %256B)."""
    gp = nc.gpsimd
    stride_bytes = elem_step * mybir.dt.size(in_ap.dtype)
    assert stride_bytes % 256 == 0
    _in_ap = gp.lower_ap_dma(in_ap, for_custom_bir_dma=True)
    _idxs_ap = gp.lower_ap(idxs_ap)
    _out_ap = gp.lower_ap(out_ap)
    return gp.add_instruction(mybir.InstDMAGatherAnt(
        name=nc.get_next_instruction_name(),
        ins=[*_in_ap, _idxs_ap, gp.lower_val_access(gp.to_reg(num_idxs))],
        outs=[_out_ap],
        transpose=False, num_idxs=num_idxs, elem_size=elem_size,
        stride_bytes_256=stride_bytes // 256, gen_mode=0,
        single_packet=False, queue_num=0,
        sbuf_tokens_per_rank=0, sbuf_free_dim_per_rank=0,
        sbuf_free_dim_pad_per_rank=0, sbuf_byte_offset=0))


def build_kernel2(meta, stage='full'):
    chunks = meta["chunks"]
    Lq = meta["Lq"]
    qblock0 = meta["qblock0"]
    NBtot = int(meta["NBtot"])
    Ltot = int(meta["Ltot"])
    qpos0 = np.concatenate([[0], np.cumsum(Lq)]).astype(np.int64)

    nc = bacc.Bacc("TRN2", target_bir_lowering=False, debug=False,
                   enable_asserts=False, num_devices=NCORES)
    AL = mybir.AluOpType
    AX = mybir.AxisListType
    ACT = mybir.ActivationFunctionType

    xT = nc.dram_tensor("xT", [F_IN, RN], bf16, kind="ExternalInput")
    W1a = nc.dram_tensor("W1a", [F_IN, 80], bf16, kind="ExternalInput")
    W2a = nc.dram_tensor("W2a", [F1, 34], bf16, kind="ExternalInput")
    cvec = nc.dram_tensor("cvec", [P, 34], f32, kind="ExternalInput")
    t1i = nc.dram_tensor("t1i", [P, Ltot // 16], i16, kind="ExternalInput")
    dcol_d = nc.dram_tensor("dcol", [P, NBtot], bf16, kind="ExternalInput")
    drow_d = nc.dram_tensor("drow", [1, NBtot * P], bf16, kind="ExternalInput")
    iotac = nc.dram_tensor("iotac", [P, 256], bf16, kind="ExternalInput")
    iotap = nc.dram_tensor("iotap", [P, 2], bf16, kind="ExternalInput")
    out = nc.dram_tensor("out", [RN, C2], f32, kind="ExternalOutput")

    run_l1 = stage in ('l1g', 'l1', 'p1', 'ag2', 'l2g', 'l2', 'full')
    run_p1 = stage in ('p1', 'ag2', 'l2g', 'l2', 'full')
    run_ag2 = stage in ('ag2', 'l2g', 'l2', 'full')
    run_l2 = stage in ('l2g', 'l2', 'full')
    run_p2 = stage in ('full',)
    run_ag1 = stage in ('ag',) or run_l1

    with tile.TileContext(nc) as tc:
        with tc.tile_pool(name="dram", bufs=1, space="DRAM") as dram, \
             tc.tile_pool(name="const", bufs=1) as cpool:
            T1c = dram.tile([RROWS, 128], bf16)
            T1f = dram.tile([NCORES * RROWS, 128], bf16, addr_space="Shared")
            T2c = dram.tile([RROWS, 128], bf16)
            T2f = dram.tile([NCORES * RROWS, 128], bf16, addr_space="Shared")

            from concourse.masks import make_identity
            ident = cpool.tile([P, P], f32)
            make_identity(nc, ident[:])
            w2sb = cpool.tile([F1, 34], bf16)
            nc.sync.dma_start(w2sb[:], W2a[:])
            cvr = cpool.tile([P, 34], f32)
            nc.sync.dma_start(cvr[:], cvec[:])
            iot = cpool.tile([P, 256], bf16)     # [:,0:128]=j, [:,128:256]=j+128
            nc.sync.dma_start(iot[:], iotac[:])
            iop = cpool.tile([P, 2], bf16)       # [:,0]=p, [:,1]=p+128
            nc.sync.dma_start(iop[:], iotap[:])
            dummy1 = cpool.tile([1, 128], bf16)
            nc.vector.memset(dummy1[:, 0:64], 0.0)
            nc.vector.memset(dummy1[:, 64:128], -1e30)
            dummy2 = cpool.tile([1, 128], bf16)
            nc.vector.memset(dummy2[:, 0:32], 0.0)
            nc.vector.memset(dummy2[:, 32:128], -1e30)
            nc.sync.dma_start(T1c[RN:RN + 1, :], dummy1[:])
            nc.sync.dma_start(T2c[RN:RN + 1, :], dummy2[:])

            idx_res = cpool.tile([P, Ltot // 16], i16)
            nc.sync.dma_start(idx_res[:], t1i[:])
            dcol = cpool.tile([P, NBtot], bf16)
            nc.sync.dma_start(dcol[:], dcol_d[:])

            adt1 = cpool.tile([P, NT, 8], bf16)
            nc.vector.memset(adt1[:].rearrange("p t e -> p (t e)"), 0.0)
            adt2 = cpool.tile([P, NT, 1], bf16)
            nc.vector.memset(adt2[:].rearrange("p t e -> p (t e)"), 0.0)
            nd1 = cpool.tile([P, NT, 72], f32)
            nc.vector.memset(nd1[:].rearrange("p t e -> p (t e)"), 0.0)
            nd2 = cpool.tile([P, NT, 33], f32)
            nc.vector.memset(nd2[:].rearrange("p t e -> p (t e)"), 0.0)

            # ---------- phase 1: h_aug = x @ W1aug ----------
            with tc.tile_pool(name="mmx", bufs=2) as xpool, \
                 tc.tile_pool(name="mmw", bufs=4) as wpool, \
                 tc.tile_pool(name="mmp", bufs=8, space="PSUM") as pspool, \
                 tc.tile_pool(name="mmo", bufs=4) as opool:
                w1sb = []
                for kc in range(4):
                    wt_ = wpool.tile([P, 80], bf16, tag=f"w1_{kc}")
                    nc.sync.dma_start(wt_[:], W1a[kc * P:(kc + 1) * P, :])
                    w1sb.append(wt_)
                BT = 8
                for b0 in range(0, NT, BT):
                    bts = list(range(b0, min(b0 + BT, NT)))
                    c0 = b0 * P
                    c1 = min(bts[-1] * P + P, RN)
                    ps = {t: pspool.tile([P, 80], f32, space="PSUM", tag="ps",
                                         name=f"ps_{t}")
                          for t in bts}
                    for kc in range(4):
                        xs = xpool.tile([P, BT * P], bf16, tag="xs")
                        nc.sync.dma_start(xs[:, 0:c1 - c0],
                                          xT[kc * P:(kc + 1) * P, c0:c1])
                        for t in bts:
                            m = min(P, RN - t * P)
                            nc.tensor.matmul(
                                ps[t][0:m, :],
                                lhsT=xs[:, t * P - c0:t * P - c0 + m],
                                rhs=w1sb[kc][:], start=(kc == 0), stop=(kc == 3))
                    for t in bts:
                        m = min(P, RN - t * P)
                        r1 = opool.tile([P, 72], bf16, tag="r1")
                        nc.vector.tensor_copy(r1[0:m, :], ps[t][0:m, 0:72])
                        nc.sync.dma_start(
                            T1c[t * P:t * P + m, 0:72], r1[0:m, :])
                        # alpha_dst resident
                        nc.vector.tensor_copy(adt1[0:m, t, :], ps[t][0:m, 72:80])
                        # self-loop init of nd1
                        asd = opool.tile([P, 8], f32, tag="asd")
                        nc.vector.tensor_tensor(
                            out=asd[0:m, :], in0=ps[t][0:m, 64:72],
                            in1=adt1[0:m, t, :], op=AL.add)
                        tmp = opool.tile([P, 8], f32, tag="lr")
                        nc.vector.tensor_scalar(
                            out=tmp[0:m, :], in0=asd[0:m, :], scalar1=NEG,
                            scalar2=None, op0=AL.mult)
                        nc.vector.tensor_tensor(
                            out=asd[0:m, :], in0=asd[0:m, :], in1=tmp[0:m, :],
                            op=AL.max)
                        nc.scalar.activation(nd1[0:m, t, 64:72], asd[0:m, :],
                                             ACT.Exp)
                        nc.vector.tensor_tensor(
                            out=nd1[0:m, t, 0:64].rearrange(
                                "p (h c) -> p h c", c=8),
                            in0=ps[t][0:m, 0:64].rearrange(
                                "p (h c) -> p h c", c=8),
                            in1=nd1[0:m, t, 64:72].unsqueeze(2)
                            .broadcast_to([m, 8, 8]),
                            op=AL.mult)

            # ---------- AllGather T1 ----------
            if run_ag1:
                nc.gpsimd.collective_compute(
                    "AllGather", AL.bypass, ins=[T1c.opt()], outs=[T1f.opt()],
                    replica_groups=[list(range(NCORES))])

            # ---------- edge phase ----------
            def edge_layer(layer, Ttbl, ndT, adT, gather_only):
                FEAT = F1 if layer == 1 else C2       # 64 / 32
                NH = H1 if layer == 1 else 1
                EL = FEAT + NH                        # 72 / 33 gathered elems
                MW = FEAT + NH                        # message width
                ELG = 72 if layer == 1 else 34        # table row payload
                with tc.tile_pool(name=f"eg{layer}", bufs=3) as eg, \
                     tc.tile_pool(name=f"es{layer}", bufs=2) as es, \
                     tc.tile_pool(name=f"em{layer}", bufs=2) as em, \
                     tc.tile_pool(name=f"ep{layer}", bufs=2, space="PSUM") as epp, \
                     tc.tile_pool(name=f"ea{layer}", bufs=4, space="PSUM") as eap:
                    for ch in chunks:
                        q = ch["q"]
                        nblk = ch["nblk"]
                        npos = ch["npos"]
                        posg = int(qpos0[q] + ch["pos0"])
                        blkg = int(qblock0[q] + ch["b0"])
                        eb = eg.tile([P, nblk, ELG], bf16, tag="eb")
                        dma_gather_raw(
                            nc, eb[:],
                            Ttbl[2 * RROWS * q:2 * RROWS * (q + 1), 0:ELG],
                            idx_res[:, posg // 16:(posg + npos) // 16],
                            npos, ELG, 128)
                        drw = eg.tile([P, nblk * P], bf16, tag="drw")
                        nc.sync.dma_start(
                            drw[:],
                            drow_d[0:1, blkg * P:(blkg + nblk) * P]
                            .broadcast_to([P, nblk * P]))
                        if gather_only:
                            continue
                        # S_T (n-part, e-free), base-tile orientation
                        st0 = es.tile([P, nblk, P], bf16, tag="st0")
                        nc.vector.tensor_tensor(
                            out=st0[:],
                            in0=iop[:, 0:1].unsqueeze(2)
                            .broadcast_to([P, nblk, P]),
                            in1=drw[:].rearrange("p (b e) -> p b e", e=P),
                            op=AL.is_equal)
                        # straddle S_T (second tile)
                        st1 = {}
                        for (b, tf, tiles) in ch["views"]:
                            if len(tiles) == 2:
                                ib = b - ch["b0"]
                                s = es.tile([P, P], bf16, tag="st1",
                                            name=f"st1_{layer}_{blkg}_{ib}")
                                nc.vector.tensor_tensor(
                                    out=s[:],
                                    in0=iop[:, 1:2].broadcast_to([P, P]),
                                    in1=drw[:, ib * P:(ib + 1) * P],
                                    op=AL.is_equal)
                                st1[ib] = s
                        # alpha_dst per edge via S_T matmuls
                        adp = eap.tile([P, nblk, NH], f32, space="PSUM",
                                       tag="adp")
                        for (b, tf, tiles) in ch["views"]:
                            ib = b - ch["b0"]
                            for j, t in enumerate(tiles):
                                lhsT = (st0[:, ib, :] if t == tf
                                        else st1[ib][:])
                                nc.tensor.matmul(
                                    adp[:, ib, :], lhsT=lhsT,
                                    rhs=adT[:, t, :],
                                    start=(j == 0), stop=(j == len(tiles) - 1))
                        # alpha -> lrelu -> exp
                        wq = em.tile([P, nblk, NH], f32, tag="wq")
                        nc.vector.tensor_tensor(
                            out=wq[:], in0=eb[:, :, FEAT:FEAT + NH],
                            in1=adp[:], op=AL.add)
                        tmp = em.tile([P, nblk * NH], f32, tag="lrt")
                        nc.vector.tensor_scalar(
                            out=tmp[:], in0=wq[:].rearrange("p b h -> p (b h)"),
                            scalar1=NEG, scalar2=None, op0=AL.mult)
                        nc.vector.tensor_tensor(
                            out=wq[:].rearrange("p b h -> p (b h)"),
                            in0=wq[:].rearrange("p b h -> p (b h)"),
                            in1=tmp[:], op=AL.max)
                        nc.scalar.activation(
                            wq[:].rearrange("p b h -> p (b h)"),
                            wq[:].rearrange("p b h -> p (b h)"), ACT.Exp)
                        # messages
                        msg = em.tile([P, nblk, MW], bf16, tag="msg")
                        nc.vector.tensor_copy(msg[:, :, FEAT:MW], wq[:])
                        if layer == 1:
                            nc.vector.tensor_tensor(
                                out=msg[:, :, 0:64].rearrange(
                                    "p b (h c) -> p b h c", c=8),
                                in0=eb[:, :, 0:64].rearrange(
                                    "p b (h c) -> p b h c", c=8),
                                in1=wq[:].unsqueeze(3)
                                .broadcast_to([P, nblk, 8, 8]),
                                op=AL.mult)
                        else:
                            nc.vector.tensor_tensor(
                                out=msg[:, :, 0:32],
                                in0=eb[:, :, 0:32],
                                in1=wq[:].broadcast_to([P, nblk, 32]),
                                op=AL.mult)
                        # S (e-part, n-free)
                        s0 = es.tile([P, nblk, P], bf16, tag="s0")
                        nc.vector.tensor_tensor(
                            out=s0[:],
                            in0=dcol[:, blkg:blkg + nblk].unsqueeze(2)
                            .broadcast_to([P, nblk, P]),
                            in1=iot[:, 0:128].unsqueeze(1)
                            .broadcast_to([P, nblk, P]),
                            op=AL.is_equal)
                        s1 = {}
                        for (b, tf, tiles) in ch["views"]:
                            if len(tiles) == 2:
                                ib = b - ch["b0"]
                                s = es.tile([P, P], bf16, tag="s1",
                                            name=f"s1_{layer}_{blkg}_{ib}")
                                nc.vector.tensor_tensor(
                                    out=s[:],
                                    in0=dcol[:, blkg + ib:blkg + ib + 1]
                                    .broadcast_to([P, P]),
                                    in1=iot[:, 128:256],
                                    op=AL.is_equal)
                                s1[ib] = s
                        # aggregate per tile
                        tviews = {}
                        for (b, tf, tiles) in ch["views"]:
                            ib = b - ch["b0"]
                            for t in tiles:
                                tviews.setdefault(t, []).append(
                                    (ib, t == tf))
                        for t in sorted(tviews):
                            vs = tviews[t]
                            pst = epp.tile([P, MW], f32, space="PSUM",
                                           tag="aggps")
                            for j, (ib, base) in enumerate(vs):
                                lhsT = s0[:, ib, :] if base else s1[ib][:]
                                nc.tensor.matmul(
                                    pst[:], lhsT=lhsT, rhs=msg[:, ib, :],
                                    start=(j == 0), stop=(j == len(vs) - 1))
                            nc.vector.tensor_tensor(
                                out=ndT[:, t, :], in0=ndT[:, t, :],
                                in1=pst[:], op=AL.add)

            if run_l1:
                edge_layer(1, T1f, nd1, adt1, stage == 'l1g')

            # ---------- post 1: normalize, ELU+1, W2 -> T2c ----------
            if run_p1:
                EC = 10
                with tc.tile_pool(name="p1", bufs=2) as ep, \
                     tc.tile_pool(name="p1p", bufs=4, space="PSUM") as epp:
                    for e0 in range(0, NT, EC):
                        Tc = min(EC, NT - e0)
                        # +eps so the tail tile's empty rows give 0*1e30=0
                        # instead of 0*inf=NaN (which would poison layer-2
                        # matmuls through adt2 via 0*NaN).
                        reci = ep.tile([P, Tc, 8], f32, tag="reci")
                        nc.vector.tensor_scalar(
                            out=reci[:], in0=nd1[:, e0:e0 + Tc, 64:72],
                            scalar1=1e-30, scalar2=None, op0=AL.add)
                        rec = ep.tile([P, Tc, 8], f32, tag="rec")
                        nc.vector.reciprocal(rec[:], reci[:])
                        h1p = ep.tile([P, Tc, 64], f32, tag="h1p")
                        nc.vector.tensor_tensor(
                            out=h1p[:].rearrange("p t (h c) -> p t h c", c=8),
                            in0=nd1[:, e0:e0 + Tc, 0:64].rearrange(
                                "p t (h c) -> p t h c", c=8),
                            in1=rec[:].unsqueeze(3).broadcast_to([P, Tc, 8, 8]),
                            op=AL.mult)
                        # ELU + 1 = relu(x) + exp(min(x, 0))
                        rl = ep.tile([P, Tc * 64], f32, tag="rl")
                        h1f = h1p[:].rearrange("p t f -> p (t f)")
                        nc.scalar.activation(rl[:], h1f, ACT.Relu)
                        nc.vector.tensor_tensor(out=h1f, in0=h1f, in1=rl[:],
                                                op=AL.subtract)
                        nc.scalar.activation(h1f, h1f, ACT.Exp)
                        nc.vector.tensor_tensor(out=h1f, in0=h1f, in1=rl[:],
                                                op=AL.add)
                        t2a = ep.tile([P, Tc, 34], f32, tag="t2a")
                        for i in range(Tc):
                            tps = epp.tile([64, P], f32, space="PSUM",
                                           tag="tps")
                            nc.tensor.transpose(tps[:], h1p[:, i, :], ident[:])
                            hT = ep.tile([64, P], bf16, tag="hT")
                            nc.vector.tensor_copy(hT[:], tps[:])
                            ps2 = epp.tile([P, 34], f32, space="PSUM",
                                           tag="ps2")
                            nc.tensor.matmul(ps2[:], lhsT=hT[:], rhs=w2sb[:],
                                             start=True, stop=True)
                            nc.vector.tensor_tensor(
                                out=t2a[:, i, :], in0=ps2[:], in1=cvr[:],
                                op=AL.add)
                        # alpha_dst2 resident + self init of nd2
                        nc.vector.tensor_copy(
                            adt2[:, e0:e0 + Tc, 0], t2a[:, :, 33])
                        asd = ep.tile([P, Tc], f32, tag="asd2")
                        nc.vector.tensor_tensor(
                            out=asd[:], in0=t2a[:, :, 32], in1=t2a[:, :, 33],
                            op=AL.add)
                        tmp = ep.tile([P, Tc], f32, tag="lr2")
                        nc.vector.tensor_scalar(
                            out=tmp[:], in0=asd[:], scalar1=NEG, scalar2=None,
                            op0=AL.mult)
                        nc.vector.tensor_tensor(out=asd[:], in0=asd[:],
                                                in1=tmp[:], op=AL.max)
                        nc.scalar.activation(nd2[:, e0:e0 + Tc, 32], asd[:],
                                             ACT.Exp)
                        nc.vector.tensor_tensor(
                            out=nd2[:, e0:e0 + Tc, 0:32],
                            in0=t2a[:, :, 0:32],
                            in1=nd2[:, e0:e0 + Tc, 32:33]
                            .broadcast_to([P, Tc, 32]),
                            op=AL.mult)
                        t2sb = ep.tile([P, Tc, 34], bf16, tag="t2sb")
                        nc.vector.tensor_copy(
                            t2sb[:].rearrange("p t e -> p (t e)"),
                            t2a[:].rearrange("p t e -> p (t e)"))
                        nfull = min((e0 + Tc) * P, RN) - e0 * P
                        tfull = nfull // P
                        if tfull:
                            nc.sync.dma_start(
                                T2c[e0 * P:e0 * P + tfull * P, 0:34]
                                .rearrange("(t p) e -> p t e", p=P),
                                t2sb[:, 0:tfull, :])
                        rem = nfull - tfull * P
                        if rem:
                            nc.sync.dma_start(
                                T2c[e0 * P + tfull * P:
                                    e0 * P + tfull * P + rem, 0:34],
                                t2sb[0:rem, tfull, :])


            # ---------- AllGather T2 + layer 2 ----------
            if run_ag2:
                nc.gpsimd.collective_compute(
                    "AllGather", AL.bypass, ins=[T2c.opt()], outs=[T2f.opt()],
                    replica_groups=[list(range(NCORES))])
            if run_l2:
                edge_layer(2, T2f, nd2, adt2, stage == 'l2g')

            # ---------- post 2: log_softmax -> out ----------
            if run_p2:
                EC = 10
                with tc.tile_pool(name="p2", bufs=2) as ep:
                    for e0 in range(0, NT, EC):
                        Tc = min(EC, NT - e0)
                        rc2i = ep.tile([P, Tc, 1], f32, tag="rc2i")
                        nc.vector.tensor_scalar(
                            out=rc2i[:], in0=nd2[:, e0:e0 + Tc, 32:33],
                            scalar1=1e-30, scalar2=None, op0=AL.add)
                        rec2 = ep.tile([P, Tc, 1], f32, tag="rec2")
                        nc.vector.reciprocal(rec2[:], rc2i[:])
                        lg = ep.tile([P, Tc, 32], f32, tag="lg")
                        nc.vector.tensor_tensor(
                            out=lg[:], in0=nd2[:, e0:e0 + Tc, 0:32],
                            in1=rec2[:].broadcast_to([P, Tc, 32]), op=AL.mult)
                        mx = ep.tile([P, Tc], f32, tag="mx")
                        nc.vector.tensor_reduce(out=mx[:], in_=lg[:],
                                                axis=AX.X, op=AL.max)
                        nc.vector.tensor_tensor(
                            out=lg[:], in0=lg[:],
                            in1=mx[:].unsqueeze(2).broadcast_to([P, Tc, 32]),
                            op=AL.subtract)
                        ex = ep.tile([P, Tc, 32], f32, tag="ex")
                        nc.scalar.activation(
                            ex[:].rearrange("p t c -> p (t c)"),
                            lg[:].rearrange("p t c -> p (t c)"), ACT.Exp)
                        sm = ep.tile([P, Tc], f32, tag="sm")
                        nc.vector.tensor_reduce(out=sm[:], in_=ex[:],
                                                axis=AX.X, op=AL.add)
                        nc.scalar.activation(sm[:], sm[:], ACT.Ln)
                        nc.vector.tensor_tensor(
                            out=lg[:], in0=lg[:],
                            in1=sm[:].unsqueeze(2).broadcast_to([P, Tc, 32]),
                            op=AL.subtract)
                        nfull = min((e0 + Tc) * P, RN) - e0 * P
                        tfull = nfull // P
                        if tfull:
                            nc.sync.dma_start(
                                out[e0 * P:e0 * P + tfull * P, :]
                                .rearrange("(t p) e -> p t e", p=P),
                                lg[:, 0:tfull, :])
                        rem = nfull - tfull * P
                        if rem:
                            nc.sync.dma_start(
                                out[e0 * P + tfull * P:
                                    e0 * P + tfull * P + rem, :],
                                lg[0:rem, tfull, :])

    nc.finalize()
    return nc


def make_in_maps(meta, percore, x, W1, a1s, a1d, W2, a2s, a2d):
    A1s = np.zeros((F1, H1), np.float32)
    A1d = np.zeros((F1, H1), np.float32)
    for h in range(H1):
        A1s[h * C1:(h + 1) * C1, h] = a1s[h]
        A1d[h * C1:(h + 1) * C1, h] = a1d[h]
    W1aug = np.concatenate([W1, W1 @ A1s, W1 @ A1d], axis=1)          # [512,80]
    W2aug = np.concatenate([W2, W2 @ a2s.reshape(C2, 1),
                            W2 @ a2d.reshape(C2, 1)], axis=1)          # [64,34]
    cvecv = np.tile((-W2aug.sum(axis=0, dtype=np.float64))
                    .astype(np.float32).reshape(1, 34), (P, 1))
    xTf = np.ascontiguousarray(x.T).astype(BF)
    W1a_bf = W1aug.astype(BF)
    W2a_bf = W2aug.astype(BF)
    iotac = np.tile(np.arange(256, dtype=np.float32).reshape(1, 256),
                    (P, 1)).astype(BF)
    iotap = np.stack([np.arange(P, dtype=np.float32),
                      np.arange(P, dtype=np.float32) + 128],
                     axis=1).astype(BF)
    in_maps = []
    for k in range(NCORES):
        pc = percore[k]
        g16 = pc["g16"]
        dval = pc["dval"].astype(BF)
        NB = len(g16) // P
        in_maps.append(dict(
            xT=np.ascontiguousarray(xTf[:, k * RN:(k + 1) * RN]),
            W1a=W1a_bf, W2a=W2a_bf, cvec=cvecv,
            t1i=_wrap_idx(g16),
            dcol=np.ascontiguousarray(dval.reshape(NB, P).T),
            drow=dval.reshape(1, -1),
            iotac=iotac, iotap=iotap))
    return in_maps


def kernel(**inputs):
    x = np.asarray(inputs["x"], np.float32)
    edge_index = np.asarray(inputs["edge_index"])
    W1 = np.asarray(inputs["W1"], np.float32)
    a1s = np.asarray(inputs["att_src1"], np.float32)
    a1d = np.asarray(inputs["att_dst1"], np.float32)
    W2 = np.asarray(inputs["W2"], np.float32)
    a2s = np.asarray(inputs["att_src2"], np.float32)
    a2d = np.asarray(inputs["att_dst2"], np.float32)

    key = hashlib.sha1(edge_index.tobytes()).hexdigest()
    if key not in _cache:
        meta, percore = host_prep2(edge_index)
        nc = build_kernel2(meta)
        _cache[key] = (meta, percore, nc)
    meta, percore, nc = _cache[key]

    in_maps = make_in_maps(meta, percore, x, W1, a1s, a1d, W2, a2s, a2d)
    global _last_in_maps
    _last_in_maps = in_maps
    res = bass_utils.run_bass_kernel_spmd(nc, in_maps,
                                          core_ids=list(range(NCORES)))
    return np.concatenate([np.asarray(res.results[k]["out"])
                           for k in range(NCORES)], axis=0)


def timed_run(n=5, depth=96):
    """Pipelined per-execution NEFF time (amortizes the host round-trip)."""
    import time
    import jax
    from jax.sharding import Mesh, PartitionSpec
    from jax.experimental.shard_map import shard_map
    from concourse import bass2jax, mybir as mb

    meta, percore, nc = next(iter(_cache.values()))
    in_maps = _last_in_maps
    bass2jax.install_neuronx_cc_hook()
    in_names, out_names, out_avals, zero_outs = [], [], [], []
    for alloc in nc.m.functions[0].allocations:
        if not isinstance(alloc, mb.MemoryLocationSet):
            continue
        name = alloc.memorylocations[0].name
        pid_name = nc.partition_id_tensor.name if nc.partition_id_tensor else None
        if alloc.kind == "ExternalInput":
            if name != pid_name:
                in_names.append(name)
        elif alloc.kind == "ExternalOutput":
            out_names.append(name)
            shape = tuple(alloc.tensor_shape)
            dtype = mb.dt.np(alloc.dtype)
            out_avals.append(jax.core.ShapedArray(shape, dtype))
            zero_outs.append(np.zeros(shape, dtype))
    n_params = len(in_names)
    all_names = in_names + out_names
    if nc.partition_id_tensor is not None:
        all_names = all_names + [nc.partition_id_tensor.name]

    def _body(*args):
        ops = list(args)
        if nc.partition_id_tensor is not None:
            ops.append(bass2jax.partition_id_tensor())
        outs = bass2jax._bass_exec_p.bind(
            *ops, out_avals=tuple(out_avals), in_names=tuple(all_names),
            out_names=tuple(out_names), lowering_input_output_aliases=(),
            sim_require_finite=True, sim_require_nnan=True, nc=nc)
        return tuple(outs)

    devices = jax.devices()[:NCORES]
    mesh = Mesh(np.asarray(devices), ("core",))
    nin = n_params + len(out_names)
    sm = shard_map(_body, mesh=mesh,
                   in_specs=(PartitionSpec("core"),) * nin,
                   out_specs=(PartitionSpec("core"),) * len(out_names),
                   check_rep=False)
    fn = jax.jit(sm, keep_unused=True)
    concat_in = [np.concatenate([np.asarray(in_maps[c][nm])
                                 for c in range(NCORES)]) for nm in in_names]
    concat_zero = [np.zeros((NCORES * z.shape[0], *z.shape[1:]), z.dtype)
                   for z in zero_outs]
    sh = jax.sharding.NamedSharding(mesh, PartitionSpec("core"))
    dev_in = [jax.device_put(a, sh) for a in concat_in + concat_zero]
    outw = fn(*dev_in)
    jax.block_until_ready(outw)
    ts = []
    for _ in range(n):
        t0 = time.perf_counter()
        outs = [fn(*dev_in) for _ in range(depth)]
        jax.block_until_ready(outs)
        ts.append((time.perf_counter() - t0) / depth)
    return min(ts) * 1e9
